# revision 14
# baseline (speedup 1.0000x reference)
"""Causal shaped attention kernel for Trainium2 (8 NeuronCores).

y = beta * softmax(causal(q k^T / 8)) @ v + alpha * Id @ v - gamma * MC @ v
  with q,k = x @ w_attn.T split, v = x, Id = softmax(eye(T)), MC = causal row-mean.

Sharding: (batch, head-group) across 8 cores: core c -> b = c//2, heads
h0 = (c%2)*8 .. h0+8.  Each core computes y[b, :, h0*64 : h0*64+512].

Id@v and MC@v have closed forms (no T x T materialization):
  Id@v[i] = ((e-1) v[i] + colsum(v)) / (e+T-1)
  MC@v[i] = cumsum(v)[i] / (i+1)

Design (v5):
 - x^T, W^T and the vones ([v|1]) AV-lhsT are packed on HOST in bf16: no
   on-device transposes, half the input DMA bytes and LDWEIGHTS traffic.
 - Phase C processes HEAD PAIRS: the two heads' K=64 QK matmuls occupy
   partitions 0:64 / 64:128, so the PE row-tiles them (tile_position
   auto-derived from lhsT base partition) and runs both concurrently --
   QK cost per pair ~= cost of one head.
 - Software pipeline at j-tile granularity: QK+exp stream of unit u+1
   interleaves with the AV stream of unit u; projection chains for head
   pair p+1 are dripped into the PE stream during pair p's units.
 - bf16 everywhere on the PE (qkT, exp output, vones); PSUM stays fp32.
 - B2 static term: one colsum + one K=1 prefix-broadcast + one tril
   matmul per 128-row tile, rolling [1,512] prefix row, all under the
   input-DMA shadow.
"""

import sys

if "/opt/trn_rl_repo" not in sys.path:
    sys.path.insert(0, "/opt/trn_rl_repo")

import math

import numpy as np
import ml_dtypes

import concourse.bass as bass
import concourse.mybir as mybir
import concourse.tile as tile
from concourse import bacc
from concourse.bass_utils import run_bass_kernel_spmd

F32 = mybir.dt.float32
BF16 = mybir.dt.bfloat16
AF = mybir.ActivationFunctionType
OP = mybir.AluOpType
BFNP = ml_dtypes.bfloat16

N_CORES = 8
B, T, C = 4, 2048, 1024
H, HD = 16, 64
NHC = 8          # heads per core
NT = T // 128    # 16 j/i tiles
NS = 4           # i-strips of 512

# bf16 consts: tril 128 | trl2 256 | zrl2 512 | ones 1 | identb 128
CB_TRIL = 0
CB_TRL2 = 128
CB_ZRL2 = 384
CB_ONEC = 896
CB_IDB = 897
CB_W = 897 + 128
# f32 consts: negipg 16 | k1 | k2 | kb | pad | ident128
CF_IDENT = 20
CF_W = 20 + 128

_NC_CACHE = {}


def emit(nc, tc, xT, WT, vones, consb, consf, yout):
    pools = []

    def pool(name, **kw):
        p = tc.alloc_tile_pool(name=name, **kw)
        pools.append(p)
        return p

    # ---- persistent SBUF ----
    cpool = pool("cpool", bufs=1)
    cb = cpool.tile([128, CB_W], BF16, name="cb")
    cf = cpool.tile([128, CF_W], F32, name="cf")
    xTs = cpool.tile([128, 8, 2048], BF16, name="xTs")
    WTs = cpool.tile([128, 8, 1024], BF16, name="WTs")
    vos = cpool.tile([128, NHC, NT, 65], BF16, name="vos")
    qkT = cpool.tile([128, 4, 2, 2048], BF16, name="qkT")
    static = cpool.tile([128, NT, 512], F32, name="static")
    colb = cpool.tile([128, 512], F32, name="colb")
    run = cpool.tile([1, 512], BF16, name="run")

    nc.sync.dma_start(out=cb[:], in_=consb[:])
    nc.sync.dma_start(out=cf[:], in_=consf[:])
    nc.sync.dma_start(out=vos[:], in_=vones[:].rearrange(
        "p (h J d) -> p h J d", h=NHC, J=NT))
    nc.sync.dma_start(out=WTs[:], in_=WT[:].rearrange("p (c d) -> p c d", c=8))
    nc.sync.dma_start(out=xTs[:], in_=xT[:].rearrange("p (c i) -> p c i", c=8))

    tril = cb[:, CB_TRIL:CB_TRIL + 128]
    trl2 = cb[:, CB_TRL2:CB_TRL2 + 256].rearrange("p (a b) -> p a b", a=2)
    zrl2 = cb[:, CB_ZRL2:CB_ZRL2 + 512].rearrange("p (a b) -> p a b", a=2)
    identf = cf[:, CF_IDENT:CF_IDENT + 128]
    ones_col = cb[:, CB_ONEC:CB_ONEC + 1]
    identb = cb[:, CB_IDB:CB_IDB + 128]
    k1c = cf[:, 16:17]
    k2c = cf[:, 17:18]
    kbc = cf[:, 18:19]

    # ---- PSUM: pj/tp pool (2 banks) + B2 (5, released) + C (6) ----
    pspj = pool("pspj", bufs=2, space="PSUM")
    psB2 = pool("psB2", bufs=1, space="PSUM")

    # ================= B2: static = k1*v + k2*colsum - g/(i+1)*cumsum ======
    ones_row = tril[0:1, 0:128]   # tril row 0 == all ones (K=1 lhsT)
    nc.vector.memset(run[:], 0.0)
    for I in range(NT):
        cp = psB2.tile([1, 512], F32, name="cp", tag="cp", bufs=2)
        nc.tensor.matmul(cp[0:1, :], ones_col, vos[:, :, I, 0:64],
                         start=True, stop=True)
        cu = psB2.tile([128, 512], F32, name="cu", tag="cu", bufs=2)
        nc.tensor.matmul(cu[:], ones_row, run[0:1, :], start=True, stop=False)
        nc.tensor.matmul(cu[:], tril, vos[:, :, I, 0:64], start=False, stop=True)
        nc.vector.tensor_add(run[0:1, :], run[0:1, :], cp[0:1, :])
        # static_I = negipg_I * cu  (cu bank recycled 2 tiles later)
        nc.vector.tensor_scalar(out=static[:, I, :], in0=cu[:],
                                scalar1=cf[:, I:I + 1], scalar2=None,
                                op0=OP.mult)
    colb_ps = psB2.tile([128, 512], F32, name="colb_ps", tag="colb")
    nc.tensor.matmul(colb_ps[:], ones_row, run[0:1, :], start=True, stop=True)
    nc.vector.tensor_scalar(out=colb[:], in0=colb_ps[:], scalar1=k2c,
                            scalar2=None, op0=OP.mult)
    for I in range(NT):
        # static_I += k1*v_I
        nc.vector.scalar_tensor_tensor(
            out=static[:, I, :].rearrange("p (h d) -> p h d", h=NHC),
            in0=vos[:, :, I, 0:64], scalar=k1c,
            in1=static[:, I, :].rearrange("p (h d) -> p h d", h=NHC),
            op0=OP.mult, op1=OP.add)
    for I in range(NT):
        # static_I += colb
        nc.vector.tensor_add(static[:, I, :], static[:, I, :], colb[:])
    psB2.release()
    pools.remove(psB2)

    # ================= projection groups =================
    def proj_group(p4, qk, s):
        def go():
            pj = pspj.tile([128, 512], F32, name="pj", tag="pj")
            for ci in range(8):
                nc.tensor.matmul(
                    pj[:], WTs[:, ci, qk * 512 + p4 * 128: qk * 512 + (p4 + 1) * 128],
                    xTs[:, ci, s * 512:(s + 1) * 512],
                    start=(ci == 0), stop=(ci == 7))
            dst = qkT[:, p4, qk, s * 512:(s + 1) * 512]
            if p4 == 0:
                nc.scalar.copy(out=dst, in_=pj[:])
            else:
                nc.vector.tensor_copy(out=dst, in_=pj[:])
        return go

    proj_sched = [[proj_group(p4, qk, s) for s in range(NS) for qk in (1, 0)]
                  for p4 in range(4)]
    proj_emitted = [0, 0, 0, 0]

    def proj_pop(p4):
        proj_sched[p4][proj_emitted[p4]]()
        proj_emitted[p4] += 1

    # first two groups of pair 0 (k and q for strip 0) up-front
    proj_pop(0)
    proj_pop(0)

    # ================= phase C: head-pair units =================
    psst = pool("psst", bufs=2, space="PSUM")   # st [128,2,512] -> 4 banks
    psyp = pool("psyp", bufs=1, space="PSUM")   # yps e + o     -> 2 banks
    ptp = pool("ptp", bufs=18)                  # pt ring (bf16)
    ysbp = pool("ysbp", bufs=2)
    rcp = pool("rcp", bufs=2)
    yop = pool("yop", bufs=2)

    units = [(p4, g) for p4 in range(4)
             for g in ([0, 1, 2, 3] if p4 < 3 else [3, 2, 1, 0])]
    pt_map = {}
    yps_map = {}

    def tile_off(g, J):
        dl = J - 4 * g
        if dl < 1:
            return 0
        return (0, 128, 256, 256)[dl]

    def qk_groups(u):
        p4, g = u
        qT = (qkT[0:64, p4, 0, :], qkT[64:128, p4, 0, :])
        kT = (qkT[0:64, p4, 1, :], qkT[64:128, p4, 1, :])
        i0 = g * 512
        nj = 4 * g + 4

        def tile_go(J):
            a = tile_off(g, J)

            def go():
                st = psst.tile([128, 2, 512], F32, name="st", tag="st")
                for idx in range(2):
                    # even head lhsT base partition 0 -> PE row-tile (0,0);
                    # odd head base 64 -> row-tile (64,0): both run together
                    nc.tensor.matmul(st[:, idx, a:512],
                                     kT[idx][:, J * 128:(J + 1) * 128],
                                     qT[idx][:, i0 + a:i0 + 512],
                                     start=True, stop=True,
                                     skip_group_check=True)
                pt2 = ptp.tile([128, 2, 512], BF16, name="pt2", tag="pt2")
                nc.scalar.activation(out=pt2[:, :, a:512], in_=st[:, :, a:512],
                                     func=AF.Exp, scale=0.125)
                dl = J - 4 * g
                if dl == 0:
                    nc.vector.tensor_mul(pt2[:, :, 0:128], pt2[:, :, 0:128], trl2)
                elif dl == 1:
                    nc.vector.tensor_mul(pt2[:, :, 128:256], pt2[:, :, 128:256],
                                         trl2)
                elif dl == 2:
                    nc.vector.tensor_mul(pt2[:, :, 256:384], pt2[:, :, 256:384],
                                         trl2)
                elif dl == 3:
                    nc.vector.tensor_mul(pt2[:, :, 256:512], pt2[:, :, 256:512],
                                         zrl2)
                pt_map[(u, J)] = pt2
            return go

        return [tile_go(J) for J in range(nj)]

    def av_groups(u):
        p4, g = u
        nj = 4 * g + 4

        def tile_go(J):
            a = tile_off(g, J)

            def go():
                if J == 0:
                    yps_map[(u, 0)] = psyp.tile([128, 512], F32, name="ypse",
                                                tag="e")
                    yps_map[(u, 1)] = psyp.tile([128, 512], F32, name="ypso",
                                                tag="o")
                pt2 = pt_map.pop((u, J))
                for idx in range(2):
                    nc.tensor.matmul(yps_map[(u, idx)][0:65, a:512],
                                     vos[:, 2 * p4 + idx, J, :],
                                     pt2[:, idx, a:512],
                                     start=(J == 0), stop=(J == nj - 1),
                                     skip_group_check=True)
            return go

        def epi(idx):
            h = 2 * p4 + idx

            def go():
                yps = yps_map.pop((u, idx))
                ysb = ysbp.tile([65, 512], BF16, name="ysb", tag="ysb")
                nc.vector.tensor_copy(out=ysb[:], in_=yps[0:65, :])
                # transposes land in a pj-pool bank (shared with proj chains),
                # bf16 via bitcast view (1 cyc/row + half LDWEIGHTS)
                tp = pspj.tile([128, 512], F32, name="tp", tag="pj")
                tpb = tp[:].bitcast(BF16)
                for k in range(4):
                    nc.tensor.transpose(tpb[:, k * 66:k * 66 + 65],
                                        ysb[:, k * 128:(k + 1) * 128],
                                        identb[0:65, 0:65])
                rc4 = rcp.tile([128, 4], F32, name="rc4", tag="rc4")
                nc.vector.reciprocal(out=rc4[:], in_=tpb[:, 64:264:66])
                nc.vector.tensor_scalar(out=rc4[:], in0=rc4[:], scalar1=kbc,
                                        scalar2=None, op0=OP.mult)
                yo = yop.tile([128, 4, 64], F32, name="yo", tag="yo")
                for k in range(4):
                    nc.vector.scalar_tensor_tensor(
                        out=yo[:, k, :], in0=tpb[:, k * 66:k * 66 + 64],
                        scalar=rc4[:, k:k + 1],
                        in1=static[:, 4 * g + k, h * 64:(h + 1) * 64],
                        op0=OP.mult, op1=OP.add)
                nc.sync.dma_start(
                    out=yout[g * 512:(g + 1) * 512, h * 64:(h + 1) * 64]
                    .rearrange("(k p) d -> p k d", p=128),
                    in_=yo[:])
            return go

        return [tile_go(J) for J in range(nj)] + [epi(0), epi(1)]

    # ---- pipelined emission ----
    slot = 0

    def maybe_proj(p4):
        nonlocal slot
        slot += 1
        if slot % 3 != 0:
            return
        lim = min(p4 + 1, 3)
        for q4 in range(lim + 1):
            if proj_emitted[q4] < 8:
                proj_pop(q4)
                return

    def drain_proj(p4, g):
        for q4 in range(p4):
            while proj_emitted[q4] < 8:
                proj_pop(q4)
        while proj_emitted[p4] < 2 * g + 2:
            proj_pop(p4)

    prev_av = []
    for u in units:
        p4, g = u
        drain_proj(p4, g)
        qk = qk_groups(u)
        n = max(len(qk), len(prev_av))
        for i in range(n):
            if i < len(qk):
                qk[i]()
            if i < len(prev_av):
                prev_av[i]()
            maybe_proj(p4)
        prev_av = av_groups(u)
    for goav in prev_av:
        goav()

    for p in reversed(pools):
        p.release()


def build_nc():
    if "nc" in _NC_CACHE:
        return _NC_CACHE["nc"]
    nc = bacc.Bacc("TRN2", target_bir_lowering=False)
    xT = nc.declare_dram_parameter("xT", [128, 8 * 2048], BF16, isOutput=False)
    WT = nc.declare_dram_parameter("WT", [128, 8 * 1024], BF16, isOutput=False)
    vones = nc.declare_dram_parameter("vones", [128, NHC * NT * 65], BF16,
                                      isOutput=False)
    consb = nc.declare_dram_parameter("consb", [128, CB_W], BF16, isOutput=False)
    consf = nc.declare_dram_parameter("consf", [128, CF_W], F32, isOutput=False)
    yout = nc.declare_dram_parameter("yout", [T, 512], F32, isOutput=True)
    with tile.TileContext(nc) as tc:
        emit(nc, tc, xT, WT, vones, consb, consf, yout)
    nc.compile()
    _NC_CACHE["nc"] = nc
    return nc


def make_consts(alpha, beta, gamma):
    D = math.e + T - 1
    k1 = alpha * (math.e - 1.0) / D
    k2 = alpha / D
    jj = np.arange(128)
    trilm = (jj[:, None] <= jj[None, :]).astype(np.float32)
    cbf = np.zeros((128, CB_W), dtype=np.float32)
    cbf[:, CB_TRIL:CB_TRIL + 128] = trilm
    cbf[:, CB_TRL2:CB_TRL2 + 128] = trilm
    cbf[:, CB_TRL2 + 128:CB_TRL2 + 256] = trilm
    # zrl2 = [0 | tril] per head: [0:128]=0, [128:256]=tril, x2
    cbf[:, CB_ZRL2 + 128:CB_ZRL2 + 256] = trilm
    cbf[:, CB_ZRL2 + 384:CB_ZRL2 + 512] = trilm
    cbf[:, CB_ONEC] = 1.0
    cbf[:, CB_IDB:CB_IDB + 128] = np.eye(128, dtype=np.float32)
    consb = cbf.astype(BFNP)
    consf = np.zeros((128, CF_W), dtype=np.float32)
    for I in range(16):
        consf[:, I] = -gamma / (128.0 * I + jj + 1.0)
    consf[:, 16] = k1
    consf[:, 17] = k2
    consf[:, 18] = beta
    consf[:, CF_IDENT:CF_IDENT + 128] = np.eye(128, dtype=np.float32)
    return consb, consf


def kernel(x, w_attn, alpha, beta, gamma, _trace=False):
    x = np.asarray(x, dtype=np.float32)
    w_attn = np.asarray(w_attn, dtype=np.float32)
    alpha = float(np.asarray(alpha))
    beta = float(np.asarray(beta))
    gamma = float(np.asarray(gamma))

    nc = build_nc()
    consb, consf = make_consts(alpha, beta, gamma)
    in_maps = []
    for c in range(N_CORES):
        b, h0 = c // 2, (c % 2) * 8
        c0 = h0 * 64
        xb = x[b]
        xT = np.ascontiguousarray(
            xb.T.reshape(8, 128, T).transpose(1, 0, 2)).astype(BFNP)
        wqk = np.concatenate([w_attn[c0:c0 + 512],
                              w_attn[C + c0:C + c0 + 512]], axis=0)
        WT = np.ascontiguousarray(
            wqk.T.reshape(8, 128, 1024).transpose(1, 0, 2)).astype(BFNP)
        vsl = xb[:, c0:c0 + 512].reshape(NT, 128, NHC, 64).transpose(1, 2, 0, 3)
        vo = np.ones((128, NHC, NT, 65), dtype=np.float32)
        vo[:, :, :, 0:64] = vsl
        in_maps.append({
            "xT": xT.reshape(128, 8 * 2048),
            "WT": WT.reshape(128, 8 * 1024),
            "vones": vo.astype(BFNP).reshape(128, NHC * NT * 65),
            "consb": consb, "consf": consf,
        })
    res = run_bass_kernel_spmd(nc, in_maps, list(range(N_CORES)), trace=_trace)
    y = np.empty((B, T, C), dtype=np.float32)
    for c in range(N_CORES):
        b, h0 = c // 2, (c % 2) * 8
        y[b, :, h0 * 64: h0 * 64 + 512] = res.results[c]["yout"]
    if _trace:
        kernel.last_exec_time_ns = res.exec_time_ns
    return y


# revision 15
# speedup vs baseline: 1.0304x; 1.0304x over previous
"""Causal shaped attention kernel for Trainium2 (8 NeuronCores).

y = beta * softmax(causal(q k^T / 8)) @ v + alpha * Id @ v - gamma * MC @ v
  with q,k = x @ w_attn.T split, v = x, Id = softmax(eye(T)), MC = causal row-mean.

Sharding: (batch, head-group) across 8 cores: core c -> b = c//2, heads
h0 = (c%2)*8 .. h0+8.  Each core computes y[b, :, h0*64 : h0*64+512].

Id@v and MC@v have closed forms (no T x T materialization):
  Id@v[i] = ((e-1) v[i] + colsum(v)) / (e+T-1)
  MC@v[i] = cumsum(v)[i] / (i+1)

Design (v5):
 - x^T, W^T and the vones ([v|1]) AV-lhsT are packed on HOST in bf16: no
   on-device transposes, half the input DMA bytes and LDWEIGHTS traffic.
 - Phase C processes HEAD PAIRS: the two heads' K=64 QK matmuls occupy
   partitions 0:64 / 64:128, so the PE row-tiles them (tile_position
   auto-derived from lhsT base partition) and runs both concurrently --
   QK cost per pair ~= cost of one head.
 - Software pipeline at j-tile granularity: QK+exp stream of unit u+1
   interleaves with the AV stream of unit u; projection chains for head
   pair p+1 are dripped into the PE stream during pair p's units.
 - bf16 everywhere on the PE (qkT, exp output, vones); PSUM stays fp32.
 - B2 static term: one colsum + one K=1 prefix-broadcast + one tril
   matmul per 128-row tile, rolling [1,512] prefix row, all under the
   input-DMA shadow.
"""

import sys

if "/opt/trn_rl_repo" not in sys.path:
    sys.path.insert(0, "/opt/trn_rl_repo")

import math

import numpy as np
import ml_dtypes

import concourse.bass as bass
import concourse.mybir as mybir
import concourse.tile as tile
from concourse import bacc
from concourse.bass_utils import run_bass_kernel_spmd

F32 = mybir.dt.float32
BF16 = mybir.dt.bfloat16
AF = mybir.ActivationFunctionType
OP = mybir.AluOpType
BFNP = ml_dtypes.bfloat16

N_CORES = 8
B, T, C = 4, 2048, 1024
H, HD = 16, 64
NHC = 8          # heads per core
NT = T // 128    # 16 j/i tiles
NS = 4           # i-strips of 512

# bf16 consts: tril 128 | trl2 256 | zrl2 512 | ones 1 | identb 128
CB_TRIL = 0
CB_TRL2 = 128
CB_ZRL2 = 384
CB_ONEC = 896
CB_IDB = 897
CB_W = 897 + 128
# f32 consts: negipg 16 | k1 | k2 | kb | pad | ident128
CF_IDENT = 20
CF_W = 20 + 128

_NC_CACHE = {}


def emit(nc, tc, xT, WT, vones, consb, consf, yout):
    pools = []

    def pool(name, **kw):
        p = tc.alloc_tile_pool(name=name, **kw)
        pools.append(p)
        return p

    # ---- persistent SBUF ----
    cpool = pool("cpool", bufs=1)
    cb = cpool.tile([128, CB_W], BF16, name="cb")
    cf = cpool.tile([128, CF_W], F32, name="cf")
    xTs = cpool.tile([128, 8, 2048], BF16, name="xTs")
    WTs = cpool.tile([128, 8, 1024], BF16, name="WTs")
    vos = cpool.tile([128, NHC, NT, 65], BF16, name="vos")
    qkT = cpool.tile([128, 4, 2, 2048], BF16, name="qkT")
    static = cpool.tile([128, NT, 512], F32, name="static")
    colb = cpool.tile([128, 512], F32, name="colb")
    run = cpool.tile([1, 512], BF16, name="run")

    vov = vones[:].rearrange("p (h J d) -> p h J d", h=NHC, J=NT)
    nc.sync.dma_start(out=vos[:, :, 0:8, :], in_=vov[:, :, 0:8, :])
    nc.sync.dma_start(out=cb[:], in_=consb[:])
    nc.sync.dma_start(out=cf[:], in_=consf[:])
    nc.sync.dma_start(out=vos[:, :, 8:16, :], in_=vov[:, :, 8:16, :])
    nc.sync.dma_start(out=WTs[:], in_=WT[:].rearrange("p (c d) -> p c d", c=8))
    nc.sync.dma_start(out=xTs[:], in_=xT[:].rearrange("p (c i) -> p c i", c=8))

    tril = cb[:, CB_TRIL:CB_TRIL + 128]
    trl2 = cb[:, CB_TRL2:CB_TRL2 + 256].rearrange("p (a b) -> p a b", a=2)
    zrl2 = cb[:, CB_ZRL2:CB_ZRL2 + 512].rearrange("p (a b) -> p a b", a=2)
    identf = cf[:, CF_IDENT:CF_IDENT + 128]
    ones_col = cb[:, CB_ONEC:CB_ONEC + 1]
    identb = cb[:, CB_IDB:CB_IDB + 128]
    k1c = cf[:, 16:17]
    k2c = cf[:, 17:18]
    kbc = cf[:, 18:19]

    # ---- PSUM: pj/tp pool (2 banks) + B2 (5, released) + C (6) ----
    pspj = pool("pspj", bufs=2, space="PSUM")
    psB2 = pool("psB2", bufs=1, space="PSUM")

    # ================= B2: static = k1*v + k2*colsum - g/(i+1)*cumsum ======
    ones_row = tril[0:1, 0:128]   # tril row 0 == all ones (K=1 lhsT)
    nc.vector.memset(run[:], 0.0)
    for I in range(NT):
        cp = psB2.tile([1, 512], F32, name="cp", tag="cp", bufs=2)
        nc.tensor.matmul(cp[0:1, :], ones_col, vos[:, :, I, 0:64],
                         start=True, stop=True)
        cu = psB2.tile([128, 512], F32, name="cu", tag="cu", bufs=2)
        nc.tensor.matmul(cu[:], ones_row, run[0:1, :], start=True, stop=False)
        nc.tensor.matmul(cu[:], tril, vos[:, :, I, 0:64], start=False, stop=True)
        nc.vector.tensor_add(run[0:1, :], run[0:1, :], cp[0:1, :])
        # static_I = negipg_I * cu  (cu bank recycled 2 tiles later)
        nc.vector.tensor_scalar(out=static[:, I, :], in0=cu[:],
                                scalar1=cf[:, I:I + 1], scalar2=None,
                                op0=OP.mult)
    colb_ps = psB2.tile([128, 512], F32, name="colb_ps", tag="colb")
    nc.tensor.matmul(colb_ps[:], ones_row, run[0:1, :], start=True, stop=True)
    nc.vector.tensor_scalar(out=colb[:], in0=colb_ps[:], scalar1=k2c,
                            scalar2=None, op0=OP.mult)
    for I in range(NT):
        # static_I += k1*v_I
        nc.vector.scalar_tensor_tensor(
            out=static[:, I, :].rearrange("p (h d) -> p h d", h=NHC),
            in0=vos[:, :, I, 0:64], scalar=k1c,
            in1=static[:, I, :].rearrange("p (h d) -> p h d", h=NHC),
            op0=OP.mult, op1=OP.add)
    for I in range(NT):
        # static_I += colb
        nc.vector.tensor_add(static[:, I, :], static[:, I, :], colb[:])
    psB2.release()
    pools.remove(psB2)

    # ================= projection groups =================
    def proj_group(p4, qk, s):
        def go():
            pj = pspj.tile([128, 512], F32, name="pj", tag="pj")
            for ci in range(8):
                nc.tensor.matmul(
                    pj[:], WTs[:, ci, qk * 512 + p4 * 128: qk * 512 + (p4 + 1) * 128],
                    xTs[:, ci, s * 512:(s + 1) * 512],
                    start=(ci == 0), stop=(ci == 7))
            dst = qkT[:, p4, qk, s * 512:(s + 1) * 512]
            if p4 == 0:
                nc.scalar.copy(out=dst, in_=pj[:])
            else:
                nc.vector.tensor_copy(out=dst, in_=pj[:])
        return go

    proj_sched = [[proj_group(p4, qk, s) for s in range(NS) for qk in (1, 0)]
                  for p4 in range(4)]
    proj_emitted = [0, 0, 0, 0]

    def proj_pop(p4):
        proj_sched[p4][proj_emitted[p4]]()
        proj_emitted[p4] += 1

    # first two groups of pair 0 (k and q for strip 0) up-front
    proj_pop(0)
    proj_pop(0)

    # ================= phase C: head-pair units =================
    psst = pool("psst", bufs=2, space="PSUM")   # st [128,2,512] -> 4 banks
    psyp = pool("psyp", bufs=1, space="PSUM")   # yps e + o     -> 2 banks
    ptp = pool("ptp", bufs=18)                  # pt ring (bf16)
    ysbp = pool("ysbp", bufs=2)
    rcp = pool("rcp", bufs=2)
    yop = pool("yop", bufs=2)

    units = [(p4, g) for p4 in range(4)
             for g in ([0, 1, 2, 3] if p4 < 3 else [3, 2, 1, 0])]
    pt_map = {}
    yps_map = {}

    def tile_off(g, J):
        dl = J - 4 * g
        if dl < 1:
            return 0
        return (0, 128, 256, 256)[dl]

    def qk_groups(u):
        p4, g = u
        qT = (qkT[0:64, p4, 0, :], qkT[64:128, p4, 0, :])
        kT = (qkT[0:64, p4, 1, :], qkT[64:128, p4, 1, :])
        i0 = g * 512
        nj = 4 * g + 4

        def tile_go(J):
            a = tile_off(g, J)

            def go():
                st = psst.tile([128, 2, 512], F32, name="st", tag="st")
                for idx in range(2):
                    # even head lhsT base partition 0 -> PE row-tile (0,0);
                    # odd head base 64 -> row-tile (64,0): both run together
                    nc.tensor.matmul(st[:, idx, a:512],
                                     kT[idx][:, J * 128:(J + 1) * 128],
                                     qT[idx][:, i0 + a:i0 + 512],
                                     start=True, stop=True,
                                     skip_group_check=True)
                pt2 = ptp.tile([128, 2, 512], BF16, name="pt2", tag="pt2")
                nc.scalar.activation(out=pt2[:, :, a:512], in_=st[:, :, a:512],
                                     func=AF.Exp, scale=0.125)
                dl = J - 4 * g
                if dl == 0:
                    nc.vector.tensor_mul(pt2[:, :, 0:128], pt2[:, :, 0:128], trl2)
                elif dl == 1:
                    nc.vector.tensor_mul(pt2[:, :, 128:256], pt2[:, :, 128:256],
                                         trl2)
                elif dl == 2:
                    nc.vector.tensor_mul(pt2[:, :, 256:384], pt2[:, :, 256:384],
                                         trl2)
                elif dl == 3:
                    nc.vector.tensor_mul(pt2[:, :, 256:512], pt2[:, :, 256:512],
                                         zrl2)
                pt_map[(u, J)] = pt2
            return go

        return [tile_go(J) for J in range(nj)]

    def av_groups(u):
        p4, g = u
        nj = 4 * g + 4

        def tile_go(J):
            a = tile_off(g, J)

            def go():
                if J == 0:
                    yps_map[(u, 0)] = psyp.tile([128, 512], F32, name="ypse",
                                                tag="e")
                    yps_map[(u, 1)] = psyp.tile([128, 512], F32, name="ypso",
                                                tag="o")
                pt2 = pt_map.pop((u, J))
                for idx in range(2):
                    nc.tensor.matmul(yps_map[(u, idx)][0:65, a:512],
                                     vos[:, 2 * p4 + idx, J, :],
                                     pt2[:, idx, a:512],
                                     start=(J == 0), stop=(J == nj - 1),
                                     skip_group_check=True)
            return go

        def epi(idx):
            h = 2 * p4 + idx

            def go():
                yps = yps_map.pop((u, idx))
                ysb = ysbp.tile([65, 512], BF16, name="ysb", tag="ysb")
                nc.vector.tensor_copy(out=ysb[:], in_=yps[0:65, :])
                # transposes land in a pj-pool bank (shared with proj chains),
                # bf16 via bitcast view (1 cyc/row + half LDWEIGHTS)
                tp = pspj.tile([128, 512], F32, name="tp", tag="pj")
                tpb = tp[:].bitcast(BF16)
                for k in range(4):
                    nc.tensor.transpose(tpb[:, k * 66:k * 66 + 65],
                                        ysb[:, k * 128:(k + 1) * 128],
                                        identb[0:65, 0:65])
                rc4 = rcp.tile([128, 4], F32, name="rc4", tag="rc4")
                nc.vector.reciprocal(out=rc4[:], in_=tpb[:, 64:264:66])
                nc.vector.tensor_scalar(out=rc4[:], in0=rc4[:], scalar1=kbc,
                                        scalar2=None, op0=OP.mult)
                if idx == 0:
                    yps_map[(u, 'yo')] = yop.tile([128, 4, 128], F32,
                                                  name="yo", tag="yo")
                yo = yps_map[(u, 'yo')]
                for k in range(4):
                    nc.vector.scalar_tensor_tensor(
                        out=yo[:, k, idx * 64:(idx + 1) * 64],
                        in0=tpb[:, k * 66:k * 66 + 64],
                        scalar=rc4[:, k:k + 1],
                        in1=static[:, 4 * g + k, h * 64:(h + 1) * 64],
                        op0=OP.mult, op1=OP.add)
                if idx == 1:
                    yps_map.pop((u, 'yo'))
                    nc.sync.dma_start(
                        out=yout[g * 512:(g + 1) * 512,
                                 p4 * 128:(p4 + 1) * 128]
                        .rearrange("(k p) d -> p k d", p=128),
                        in_=yo[:])
            return go

        return [tile_go(J) for J in range(nj)] + [epi(0), epi(1)]

    # ---- pipelined emission ----
    slot = 0

    def maybe_proj(p4):
        nonlocal slot
        slot += 1
        if slot % 3 != 0:
            return
        lim = min(p4 + 1, 3)
        for q4 in range(lim + 1):
            if proj_emitted[q4] < 8:
                proj_pop(q4)
                return

    def drain_proj(p4, g):
        for q4 in range(p4):
            while proj_emitted[q4] < 8:
                proj_pop(q4)
        while proj_emitted[p4] < 2 * g + 2:
            proj_pop(p4)

    prev_av = []
    for u in units:
        p4, g = u
        drain_proj(p4, g)
        qk = qk_groups(u)
        n = max(len(qk), len(prev_av))
        for i in range(n):
            if i < len(prev_av):
                prev_av[i]()
            if i < len(qk):
                qk[i]()
            maybe_proj(p4)
        prev_av = av_groups(u)
    for goav in prev_av:
        goav()

    for p in reversed(pools):
        p.release()


def build_nc():
    if "nc" in _NC_CACHE:
        return _NC_CACHE["nc"]
    nc = bacc.Bacc("TRN2", target_bir_lowering=False)
    xT = nc.declare_dram_parameter("xT", [128, 8 * 2048], BF16, isOutput=False)
    WT = nc.declare_dram_parameter("WT", [128, 8 * 1024], BF16, isOutput=False)
    vones = nc.declare_dram_parameter("vones", [128, NHC * NT * 65], BF16,
                                      isOutput=False)
    consb = nc.declare_dram_parameter("consb", [128, CB_W], BF16, isOutput=False)
    consf = nc.declare_dram_parameter("consf", [128, CF_W], F32, isOutput=False)
    yout = nc.declare_dram_parameter("yout", [T, 512], F32, isOutput=True)
    with tile.TileContext(nc) as tc:
        emit(nc, tc, xT, WT, vones, consb, consf, yout)
    nc.compile()
    _NC_CACHE["nc"] = nc
    return nc


def make_consts(alpha, beta, gamma):
    D = math.e + T - 1
    k1 = alpha * (math.e - 1.0) / D
    k2 = alpha / D
    jj = np.arange(128)
    trilm = (jj[:, None] <= jj[None, :]).astype(np.float32)
    cbf = np.zeros((128, CB_W), dtype=np.float32)
    cbf[:, CB_TRIL:CB_TRIL + 128] = trilm
    cbf[:, CB_TRL2:CB_TRL2 + 128] = trilm
    cbf[:, CB_TRL2 + 128:CB_TRL2 + 256] = trilm
    # zrl2 = [0 | tril] per head: [0:128]=0, [128:256]=tril, x2
    cbf[:, CB_ZRL2 + 128:CB_ZRL2 + 256] = trilm
    cbf[:, CB_ZRL2 + 384:CB_ZRL2 + 512] = trilm
    cbf[:, CB_ONEC] = 1.0
    cbf[:, CB_IDB:CB_IDB + 128] = np.eye(128, dtype=np.float32)
    consb = cbf.astype(BFNP)
    consf = np.zeros((128, CF_W), dtype=np.float32)
    for I in range(16):
        consf[:, I] = -gamma / (128.0 * I + jj + 1.0)
    consf[:, 16] = k1
    consf[:, 17] = k2
    consf[:, 18] = beta
    consf[:, CF_IDENT:CF_IDENT + 128] = np.eye(128, dtype=np.float32)
    return consb, consf


def kernel(x, w_attn, alpha, beta, gamma, _trace=False):
    x = np.asarray(x, dtype=np.float32)
    w_attn = np.asarray(w_attn, dtype=np.float32)
    alpha = float(np.asarray(alpha))
    beta = float(np.asarray(beta))
    gamma = float(np.asarray(gamma))

    nc = build_nc()
    consb, consf = make_consts(alpha, beta, gamma)
    in_maps = []
    for c in range(N_CORES):
        b, h0 = c // 2, (c % 2) * 8
        c0 = h0 * 64
        xb = x[b]
        xT = np.ascontiguousarray(
            xb.T.reshape(8, 128, T).transpose(1, 0, 2)).astype(BFNP)
        wqk = np.concatenate([w_attn[c0:c0 + 512],
                              w_attn[C + c0:C + c0 + 512]], axis=0)
        WT = np.ascontiguousarray(
            wqk.T.reshape(8, 128, 1024).transpose(1, 0, 2)).astype(BFNP)
        vsl = xb[:, c0:c0 + 512].reshape(NT, 128, NHC, 64).transpose(1, 2, 0, 3)
        vo = np.ones((128, NHC, NT, 65), dtype=np.float32)
        vo[:, :, :, 0:64] = vsl
        in_maps.append({
            "xT": xT.reshape(128, 8 * 2048),
            "WT": WT.reshape(128, 8 * 1024),
            "vones": vo.astype(BFNP).reshape(128, NHC * NT * 65),
            "consb": consb, "consf": consf,
        })
    res = run_bass_kernel_spmd(nc, in_maps, list(range(N_CORES)), trace=_trace)
    y = np.empty((B, T, C), dtype=np.float32)
    for c in range(N_CORES):
        b, h0 = c // 2, (c % 2) * 8
        y[b, :, h0 * 64: h0 * 64 + 512] = res.results[c]["yout"]
    if _trace:
        kernel.last_exec_time_ns = res.exec_time_ns
    return y


# revision 17
# speedup vs baseline: 1.0394x; 1.0088x over previous
"""Causal shaped attention kernel for Trainium2 (8 NeuronCores).

y = beta * softmax(causal(q k^T / 8)) @ v + alpha * Id @ v - gamma * MC @ v
  with q,k = x @ w_attn.T split, v = x, Id = softmax(eye(T)), MC = causal row-mean.

Sharding: (batch, head-group) across 8 cores: core c -> b = c//2, heads
h0 = (c%2)*8 .. h0+8.  Each core computes y[b, :, h0*64 : h0*64+512].

Id@v and MC@v have closed forms (no T x T materialization):
  Id@v[i] = ((e-1) v[i] + colsum(v)) / (e+T-1)
  MC@v[i] = cumsum(v)[i] / (i+1)

Design (v5):
 - x^T, W^T and the vones ([v|1]) AV-lhsT are packed on HOST in bf16: no
   on-device transposes, half the input DMA bytes and LDWEIGHTS traffic.
 - Phase C processes HEAD PAIRS: the two heads' K=64 QK matmuls occupy
   partitions 0:64 / 64:128, so the PE row-tiles them (tile_position
   auto-derived from lhsT base partition) and runs both concurrently --
   QK cost per pair ~= cost of one head.
 - Software pipeline at j-tile granularity: QK+exp stream of unit u+1
   interleaves with the AV stream of unit u; projection chains for head
   pair p+1 are dripped into the PE stream during pair p's units.
 - bf16 everywhere on the PE (qkT, exp output, vones); PSUM stays fp32.
 - B2 static term: one colsum + one K=1 prefix-broadcast + one tril
   matmul per 128-row tile, rolling [1,512] prefix row, all under the
   input-DMA shadow.
"""

import sys

if "/opt/trn_rl_repo" not in sys.path:
    sys.path.insert(0, "/opt/trn_rl_repo")

import math

import numpy as np
import ml_dtypes

import concourse.bass as bass
import concourse.mybir as mybir
import concourse.tile as tile
from concourse import bacc
from concourse.bass_utils import run_bass_kernel_spmd

F32 = mybir.dt.float32
BF16 = mybir.dt.bfloat16
AF = mybir.ActivationFunctionType
OP = mybir.AluOpType
BFNP = ml_dtypes.bfloat16

N_CORES = 8
B, T, C = 4, 2048, 1024
H, HD = 16, 64
NHC = 8          # heads per core
NT = T // 128    # 16 j/i tiles
NS = 4           # i-strips of 512

# bf16 consts: tril 128 | trl2 256 | zrl2 512 | ones 1 | identb 128
CB_TRIL = 0
CB_TRL2 = 128
CB_ZRL2 = 384
CB_ONEC = 896
CB_IDB = 897
CB_W = 897 + 128
# f32 consts: negipg 16 | k1 | k2 | kb | pad | ident128
CF_IDENT = 20
CF_W = 20 + 128

_NC_CACHE = {}


def emit(nc, tc, xT, WT, vones, consb, consf, yout):
    pools = []

    def pool(name, **kw):
        p = tc.alloc_tile_pool(name=name, **kw)
        pools.append(p)
        return p

    # ---- persistent SBUF ----
    cpool = pool("cpool", bufs=1)
    cb = cpool.tile([128, CB_W], BF16, name="cb")
    cf = cpool.tile([128, CF_W], F32, name="cf")
    xTs = cpool.tile([128, 8, 2048], BF16, name="xTs")
    WTs = cpool.tile([128, 8, 1024], BF16, name="WTs")
    vos = cpool.tile([128, NHC, NT, 65], BF16, name="vos")
    qkT = cpool.tile([128, 4, 2, 2048], BF16, name="qkT")
    static = cpool.tile([128, NT, 512], F32, name="static")
    colb = cpool.tile([128, 512], F32, name="colb")
    run = cpool.tile([1, 512], BF16, name="run")

    vov = vones[:].rearrange("p (h J d) -> p h J d", h=NHC, J=NT)
    nc.sync.dma_start(out=vos[:, :, 0:8, :], in_=vov[:, :, 0:8, :])
    nc.sync.dma_start(out=cb[:], in_=consb[:])
    nc.sync.dma_start(out=cf[:], in_=consf[:])
    nc.sync.dma_start(out=vos[:, :, 8:16, :], in_=vov[:, :, 8:16, :])
    nc.sync.dma_start(out=WTs[:], in_=WT[:].rearrange("p (c d) -> p c d", c=8))
    nc.sync.dma_start(out=xTs[:], in_=xT[:].rearrange("p (c i) -> p c i", c=8))

    tril = cb[:, CB_TRIL:CB_TRIL + 128]
    trl2 = cb[:, CB_TRL2:CB_TRL2 + 256].rearrange("p (a b) -> p a b", a=2)
    zrl2 = cb[:, CB_ZRL2:CB_ZRL2 + 512].rearrange("p (a b) -> p a b", a=2)
    identf = cf[:, CF_IDENT:CF_IDENT + 128]
    ones_col = cb[:, CB_ONEC:CB_ONEC + 1]
    identb = cb[:, CB_IDB:CB_IDB + 128]
    k1c = cf[:, 16:17]
    k2c = cf[:, 17:18]
    kbc = cf[:, 18:19]

    # ---- PSUM: pj/tp pool (2 banks) + B2 (5, released) + C (6) ----
    pspj = pool("pspj", bufs=2, space="PSUM")
    psB2 = pool("psB2", bufs=1, space="PSUM")

    # ================= B2: static = k1*v + k2*colsum - g/(i+1)*cumsum ======
    ones_row = tril[0:1, 0:128]   # tril row 0 == all ones (K=1 lhsT)
    nc.vector.memset(run[:], 0.0)
    for I in range(NT):
        cp = psB2.tile([1, 512], F32, name="cp", tag="cp", bufs=2)
        nc.tensor.matmul(cp[0:1, :], ones_col, vos[:, :, I, 0:64],
                         start=True, stop=True)
        cu = psB2.tile([128, 512], F32, name="cu", tag="cu", bufs=2)
        nc.tensor.matmul(cu[:], ones_row, run[0:1, :], start=True, stop=False)
        nc.tensor.matmul(cu[:], tril, vos[:, :, I, 0:64], start=False, stop=True)
        nc.vector.tensor_add(run[0:1, :], run[0:1, :], cp[0:1, :])
        # static_I = negipg_I * cu  (cu bank recycled 2 tiles later)
        nc.vector.tensor_scalar(out=static[:, I, :], in0=cu[:],
                                scalar1=cf[:, I:I + 1], scalar2=None,
                                op0=OP.mult)
    colb_ps = psB2.tile([128, 512], F32, name="colb_ps", tag="colb")
    nc.tensor.matmul(colb_ps[:], ones_row, run[0:1, :], start=True, stop=True)
    nc.vector.tensor_scalar(out=colb[:], in0=colb_ps[:], scalar1=k2c,
                            scalar2=None, op0=OP.mult)
    for I in range(NT):
        # static_I += k1*v_I
        nc.vector.scalar_tensor_tensor(
            out=static[:, I, :].rearrange("p (h d) -> p h d", h=NHC),
            in0=vos[:, :, I, 0:64], scalar=k1c,
            in1=static[:, I, :].rearrange("p (h d) -> p h d", h=NHC),
            op0=OP.mult, op1=OP.add)
    for I in range(NT):
        # static_I += colb
        nc.vector.tensor_add(static[:, I, :], static[:, I, :], colb[:])
    psB2.release()
    pools.remove(psB2)

    # ================= projection groups =================
    def proj_group(p4, qk, s):
        def go():
            pj = pspj.tile([128, 512], F32, name="pj", tag="pj")
            for ci in range(8):
                nc.tensor.matmul(
                    pj[:], WTs[:, ci, qk * 512 + p4 * 128: qk * 512 + (p4 + 1) * 128],
                    xTs[:, ci, s * 512:(s + 1) * 512],
                    start=(ci == 0), stop=(ci == 7))
            dst = qkT[:, p4, qk, s * 512:(s + 1) * 512]
            if p4 == 0:
                nc.scalar.copy(out=dst, in_=pj[:])
            else:
                nc.vector.tensor_copy(out=dst, in_=pj[:])
        return go

    proj_sched = [[proj_group(p4, qk, s) for s in range(NS) for qk in (1, 0)]
                  for p4 in range(4)]
    proj_emitted = [0, 0, 0, 0]

    def proj_pop(p4):
        proj_sched[p4][proj_emitted[p4]]()
        proj_emitted[p4] += 1

    # first two groups of pair 0 (k and q for strip 0) up-front
    proj_pop(0)
    proj_pop(0)

    # ================= phase C: head-pair units =================
    psst = pool("psst", bufs=2, space="PSUM")   # st [128,2,512] -> 4 banks
    psyp = pool("psyp", bufs=1, space="PSUM")   # yps e + o     -> 2 banks
    ptp = pool("ptp", bufs=18)                  # pt ring (bf16)
    ysbp = pool("ysbp", bufs=2)
    rcp = pool("rcp", bufs=2)
    yop = pool("yop", bufs=2)

    units = [(p4, g) for p4 in range(4)
             for g in ([0, 1, 2, 3] if p4 < 3 else [3, 2, 1, 0])]
    pt_map = {}
    yps_map = {}

    def tile_off(g, J):
        dl = J - 4 * g
        if dl < 1:
            return 0
        return (0, 128, 256, 256)[dl]

    def qk_groups(u):
        p4, g = u
        qT = (qkT[0:64, p4, 0, :], qkT[64:128, p4, 0, :])
        kT = (qkT[0:64, p4, 1, :], qkT[64:128, p4, 1, :])
        i0 = g * 512
        nj = 4 * g + 4

        def tile_go(J):
            a = tile_off(g, J)

            def go():
                st = psst.tile([128, 2, 512], F32, name="st", tag="st")
                for idx in range(2):
                    # even head lhsT base partition 0 -> PE row-tile (0,0);
                    # odd head base 64 -> row-tile (64,0): both run together
                    nc.tensor.matmul(st[:, idx, a:512],
                                     kT[idx][:, J * 128:(J + 1) * 128],
                                     qT[idx][:, i0 + a:i0 + 512],
                                     start=True, stop=True,
                                     skip_group_check=True)
                pt2 = ptp.tile([128, 2, 512], BF16, name="pt2", tag="pt2")
                nc.scalar.activation(out=pt2[:, :, a:512], in_=st[:, :, a:512],
                                     func=AF.Exp, scale=0.125)
                dl = J - 4 * g
                if dl == 0:
                    nc.vector.tensor_mul(pt2[:, :, 0:128], pt2[:, :, 0:128], trl2)
                elif dl == 1:
                    nc.vector.tensor_mul(pt2[:, :, 128:256], pt2[:, :, 128:256],
                                         trl2)
                pt_map[(u, J)] = pt2
            return go

        def diagB_go():
            # J2=4g+2 (i 256:512 -> cols 0:256), J3=4g+3 (i 256:512 -> 256:512)
            # share one st tile: one exp covers all four (J, head) outputs.
            J2, J3 = 4 * g + 2, 4 * g + 3

            def go():
                st = psst.tile([128, 2, 512], F32, name="st", tag="st")
                for idx in range(2):
                    nc.tensor.matmul(st[:, idx, 0:256],
                                     kT[idx][:, J2 * 128:(J2 + 1) * 128],
                                     qT[idx][:, i0 + 256:i0 + 512],
                                     start=True, stop=True,
                                     skip_group_check=True)
                for idx in range(2):
                    nc.tensor.matmul(st[:, idx, 256:512],
                                     kT[idx][:, J3 * 128:(J3 + 1) * 128],
                                     qT[idx][:, i0 + 256:i0 + 512],
                                     start=True, stop=True,
                                     skip_group_check=True)
                pt2 = ptp.tile([128, 2, 512], BF16, name="pt2", tag="pt2")
                nc.scalar.activation(out=pt2[:], in_=st[:], func=AF.Exp,
                                     scale=0.125)
                nc.vector.tensor_mul(pt2[:, :, 0:128], pt2[:, :, 0:128], trl2)
                nc.vector.tensor_mul(pt2[:, :, 256:512], pt2[:, :, 256:512],
                                     zrl2)
                pt_map[(u, J2)] = pt2
            return go

        return [tile_go(J) for J in range(4 * g + 2)] + [diagB_go()]

    def av_groups(u):
        p4, g = u
        nj = 4 * g + 4

        def tile_go(J):
            a = tile_off(g, J)

            def go():
                if J == 0:
                    yps_map[(u, 0)] = psyp.tile([128, 512], F32, name="ypse",
                                                tag="e")
                    yps_map[(u, 1)] = psyp.tile([128, 512], F32, name="ypso",
                                                tag="o")
                pt2 = pt_map.pop((u, J))
                for idx in range(2):
                    nc.tensor.matmul(yps_map[(u, idx)][0:65, a:512],
                                     vos[:, 2 * p4 + idx, J, :],
                                     pt2[:, idx, a:512],
                                     start=(J == 0), stop=False,
                                     skip_group_check=True)
            return go

        def diagB_av():
            J2, J3 = 4 * g + 2, 4 * g + 3

            def go():
                pt2 = pt_map.pop((u, J2))
                for idx in range(2):
                    nc.tensor.matmul(yps_map[(u, idx)][0:65, 256:512],
                                     vos[:, 2 * p4 + idx, J2, :],
                                     pt2[:, idx, 0:256],
                                     start=False, stop=False,
                                     skip_group_check=True)
                for idx in range(2):
                    nc.tensor.matmul(yps_map[(u, idx)][0:65, 256:512],
                                     vos[:, 2 * p4 + idx, J3, :],
                                     pt2[:, idx, 256:512],
                                     start=False, stop=True,
                                     skip_group_check=True)
            return go

        def epi(idx):
            h = 2 * p4 + idx

            def go():
                yps = yps_map.pop((u, idx))
                ysb = ysbp.tile([65, 512], BF16, name="ysb", tag="ysb")
                nc.vector.tensor_copy(out=ysb[:], in_=yps[0:65, :])
                # transposes land in a pj-pool bank (shared with proj chains),
                # bf16 via bitcast view (1 cyc/row + half LDWEIGHTS)
                tp = pspj.tile([128, 512], F32, name="tp", tag="pj")
                tpb = tp[:].bitcast(BF16)
                for k in range(4):
                    nc.tensor.transpose(tpb[:, k * 66:k * 66 + 65],
                                        ysb[:, k * 128:(k + 1) * 128],
                                        identb[0:65, 0:65])
                rc4 = rcp.tile([128, 4], F32, name="rc4", tag="rc4")
                nc.vector.reciprocal(out=rc4[:], in_=tpb[:, 64:264:66])
                nc.vector.tensor_scalar(out=rc4[:], in0=rc4[:], scalar1=kbc,
                                        scalar2=None, op0=OP.mult)
                if idx == 0:
                    yps_map[(u, 'yo')] = yop.tile([128, 4, 128], F32,
                                                  name="yo", tag="yo")
                yo = yps_map[(u, 'yo')]
                for k in range(4):
                    nc.vector.scalar_tensor_tensor(
                        out=yo[:, k, idx * 64:(idx + 1) * 64],
                        in0=tpb[:, k * 66:k * 66 + 64],
                        scalar=rc4[:, k:k + 1],
                        in1=static[:, 4 * g + k, h * 64:(h + 1) * 64],
                        op0=OP.mult, op1=OP.add)
                if idx == 1:
                    yps_map.pop((u, 'yo'))
                    for k in range(4):
                        nc.sync.dma_start(
                            out=yout[g * 512 + k * 128:g * 512 + (k + 1) * 128,
                                     p4 * 128:(p4 + 1) * 128],
                            in_=yo[:, k, :])
            return go

        return ([tile_go(J) for J in range(4 * g + 2)] + [diagB_av()]
                + [epi(0), epi(1)])

    # ---- pipelined emission ----
    slot = 0

    def maybe_proj(p4):
        nonlocal slot
        slot += 1
        if slot % 3 != 0:
            return
        lim = min(p4 + 1, 3)
        for q4 in range(lim + 1):
            if proj_emitted[q4] < 8:
                proj_pop(q4)
                return

    def drain_proj(p4, g):
        for q4 in range(p4):
            while proj_emitted[q4] < 8:
                proj_pop(q4)
        while proj_emitted[p4] < 2 * g + 2:
            proj_pop(p4)

    prev_av = []
    for u in units:
        p4, g = u
        drain_proj(p4, g)
        qk = qk_groups(u)
        n = max(len(qk), len(prev_av))
        for i in range(n):
            if i < len(prev_av):
                prev_av[i]()
            if i < len(qk):
                qk[i]()
            maybe_proj(p4)
        prev_av = av_groups(u)
    for goav in prev_av:
        goav()

    for p in reversed(pools):
        p.release()


def build_nc():
    if "nc" in _NC_CACHE:
        return _NC_CACHE["nc"]
    nc = bacc.Bacc("TRN2", target_bir_lowering=False)
    xT = nc.declare_dram_parameter("xT", [128, 8 * 2048], BF16, isOutput=False)
    WT = nc.declare_dram_parameter("WT", [128, 8 * 1024], BF16, isOutput=False)
    vones = nc.declare_dram_parameter("vones", [128, NHC * NT * 65], BF16,
                                      isOutput=False)
    consb = nc.declare_dram_parameter("consb", [128, CB_W], BF16, isOutput=False)
    consf = nc.declare_dram_parameter("consf", [128, CF_W], F32, isOutput=False)
    yout = nc.declare_dram_parameter("yout", [T, 512], F32, isOutput=True)
    with tile.TileContext(nc) as tc:
        emit(nc, tc, xT, WT, vones, consb, consf, yout)
    nc.compile()
    _NC_CACHE["nc"] = nc
    return nc


def make_consts(alpha, beta, gamma):
    D = math.e + T - 1
    k1 = alpha * (math.e - 1.0) / D
    k2 = alpha / D
    jj = np.arange(128)
    trilm = (jj[:, None] <= jj[None, :]).astype(np.float32)
    cbf = np.zeros((128, CB_W), dtype=np.float32)
    cbf[:, CB_TRIL:CB_TRIL + 128] = trilm
    cbf[:, CB_TRL2:CB_TRL2 + 128] = trilm
    cbf[:, CB_TRL2 + 128:CB_TRL2 + 256] = trilm
    # zrl2 = [0 | tril] per head: [0:128]=0, [128:256]=tril, x2
    cbf[:, CB_ZRL2 + 128:CB_ZRL2 + 256] = trilm
    cbf[:, CB_ZRL2 + 384:CB_ZRL2 + 512] = trilm
    cbf[:, CB_ONEC] = 1.0
    cbf[:, CB_IDB:CB_IDB + 128] = np.eye(128, dtype=np.float32)
    consb = cbf.astype(BFNP)
    consf = np.zeros((128, CF_W), dtype=np.float32)
    for I in range(16):
        consf[:, I] = -gamma / (128.0 * I + jj + 1.0)
    consf[:, 16] = k1
    consf[:, 17] = k2
    consf[:, 18] = beta
    consf[:, CF_IDENT:CF_IDENT + 128] = np.eye(128, dtype=np.float32)
    return consb, consf


def kernel(x, w_attn, alpha, beta, gamma, _trace=False):
    x = np.asarray(x, dtype=np.float32)
    w_attn = np.asarray(w_attn, dtype=np.float32)
    alpha = float(np.asarray(alpha))
    beta = float(np.asarray(beta))
    gamma = float(np.asarray(gamma))

    nc = build_nc()
    consb, consf = make_consts(alpha, beta, gamma)
    in_maps = []
    for c in range(N_CORES):
        b, h0 = c // 2, (c % 2) * 8
        c0 = h0 * 64
        xb = x[b]
        xT = np.ascontiguousarray(
            xb.T.reshape(8, 128, T).transpose(1, 0, 2)).astype(BFNP)
        wqk = np.concatenate([w_attn[c0:c0 + 512],
                              w_attn[C + c0:C + c0 + 512]], axis=0)
        WT = np.ascontiguousarray(
            wqk.T.reshape(8, 128, 1024).transpose(1, 0, 2)).astype(BFNP)
        vsl = xb[:, c0:c0 + 512].reshape(NT, 128, NHC, 64).transpose(1, 2, 0, 3)
        vo = np.ones((128, NHC, NT, 65), dtype=np.float32)
        vo[:, :, :, 0:64] = vsl
        in_maps.append({
            "xT": xT.reshape(128, 8 * 2048),
            "WT": WT.reshape(128, 8 * 1024),
            "vones": vo.astype(BFNP).reshape(128, NHC * NT * 65),
            "consb": consb, "consf": consf,
        })
    res = run_bass_kernel_spmd(nc, in_maps, list(range(N_CORES)), trace=_trace)
    y = np.empty((B, T, C), dtype=np.float32)
    for c in range(N_CORES):
        b, h0 = c // 2, (c % 2) * 8
        y[b, :, h0 * 64: h0 * 64 + 512] = res.results[c]["yout"]
    if _trace:
        kernel.last_exec_time_ns = res.exec_time_ns
    return y
